# revision 16
# baseline (speedup 1.0000x reference)
"""ExpanderGIN message-passing kernel for 8 Trainium2 NeuronCores.

out = relu((x + segment_sum(x[src], dst)) @ W.T + b)

Strategy (graph-parallel, no collectives):
  - Destination nodes are sharded 8 ways (12500 nodes/core -> 98 tiles of
    128 slots, degree-balanced by serpentine assignment so every tile has
    roughly equal incoming-edge count). x is replicated per core.
  - Edge rows are fetched with the SWDGE dma_gather custom instruction
    (hundreds of 512B row-descriptors per instruction, amortizing the ~1us
    descriptor-generation overhead). Its int16 index limit is handled by
    splitting x into 4 quarter-tables of 25000 rows; each (tile, quarter)
    edge sublist is padded to a multiple of 128 and gathers are batched
    over groups of 7 tiles (4 instructions per group).
  - Aggregation: for each 128-edge chunk, a one-hot(dst) matrix [128 edges,
    128 slots] is built by comparing an iota row against per-edge dst
    columns (one broadcast tensor_tensor per gather batch), then TensorE
    computes agg^T += xs^T @ onehot in PSUM (f32).
  - The self term x is added from a host-side permuted/transposed copy of
    x (sequential DMA), fused into the PSUM->SBUF eviction add.
  - agg^T [feat, nodes] feeds the MLP matmul directly as the stationary
    operand: psum_out[nodes, outfeat] = h^T.T @ W^T, plus a K=1 matmul
    adding the bias row, then ReLU on the scalar engine and DMA out.
"""

import numpy as np

N = 100000
E = 625000
D = 128
NC = 8            # cores
NPC = N // NC     # 12500 nodes per core
P = 128
TPC = (NPC + P - 1) // P   # 98 tiles per core
SLOTS = TPC * P            # 12544 slots per core
NQ = 4                     # quarter tables (int16 index limit)
QROWS = N // NQ            # 25000
MAXB = 8                   # dma_gather limit: 1024 indices = 8 blocks per instruction

_f32 = np.float32


def _preprocess(edge_index):
    """Shard edges. Returns per-core host arrays + layout metadata."""
    src = np.asarray(edge_index[0]).astype(np.int64)
    dst = np.asarray(edge_index[1]).astype(np.int64)
    deg = np.bincount(dst, minlength=N)

    # serpentine degree-balanced node -> slot assignment per core
    node_of = np.full((NC, SLOTS), -1, np.int64)   # slot -> global node
    slot_of = np.empty(N, np.int64)                # global node -> slot (in its core)
    for c in range(NC):
        nodes = np.arange(c * NPC, (c + 1) * NPC)
        order = nodes[np.argsort(-deg[nodes], kind="stable")]
        padded = np.concatenate([order, np.full(SLOTS - NPC, -1, np.int64)])
        arr = padded.reshape(P, TPC).copy()
        arr[1::2] = arr[1::2, ::-1]
        node_of[c] = arr.T.reshape(-1)
        m = node_of[c] >= 0
        slot_of[node_of[c][m]] = np.nonzero(m)[0]

    ec = dst // NPC
    eslot = slot_of[dst]
    et = eslot // P
    epos = (eslot % P).astype(_f32)
    eq = src // QROWS
    eqidx = (src % QROWS).astype(np.int16)

    # counts per (core, tile, quarter); block counts = max over cores
    key = (ec * TPC + et) * NQ + eq
    cnt = np.bincount(key, minlength=NC * TPC * NQ).reshape(NC, TPC, NQ)
    Btq = ((cnt.max(axis=0) + P - 1) // P).astype(np.int64)  # [TPC, NQ]

    # pack consecutive tiles into gather groups: every quarter's block sum <= MAXB
    groups = []
    t0 = 0
    while t0 < TPC:
        t1 = t0 + 1
        while t1 < TPC and all(
            Btq[t0:t1 + 1, q].sum() <= MAXB for q in range(NQ)
        ):
            t1 += 1
        groups.append((t0, t1))
        t0 = t1

    # slot layout: nest group -> q -> t in group -> blocks
    slot_start = np.zeros((TPC, NQ), np.int64)
    pos = 0
    for (ta, tb) in groups:
        for q in range(NQ):
            for t in range(ta, tb):
                slot_start[t, q] = pos
                pos += Btq[t, q] * P
    S_total = pos
    assert S_total % 128 == 0

    # rank of each edge within its (c,t,q) group
    perm = np.argsort(key, kind="stable")
    gstart = np.concatenate([[0], np.cumsum(np.bincount(key, minlength=NC * TPC * NQ))])[:-1]
    ranks = np.empty(len(perm), np.int64)
    ranks[perm] = np.arange(len(perm)) - gstart[key[perm]]

    flat = slot_start[et, eq] + ranks   # slot within core's flat layout

    qidx_slots = np.zeros((NC, S_total), np.int16)
    dst_slots = np.full((NC, S_total), 999.0, _f32)
    qidx_slots[ec, flat] = eqidx
    dst_slots[ec, flat] = epos

    idx16 = np.empty((NC, P, S_total // 16), np.int16)
    dstl = np.empty((NC, P, S_total // 128), _f32)
    for c in range(NC):
        wrapped = qidx_slots[c].reshape(-1, 16).T   # [16, S/16]
        idx16[c] = np.tile(wrapped, (8, 1))
        dstl[c] = dst_slots[c].reshape(-1, 128).T   # [128, S/128]

    return {
        "Btq": Btq,
        "slot_start": slot_start,
        "S_total": S_total,
        "groups": groups,
        "idx16": idx16,
        "dstl": dstl,
        "node_of": node_of,
    }


def _build_program(Btq, slot_start, S_total, groups, repeat=1):
    import concourse.bacc as bacc
    import concourse.mybir as mybir
    import concourse.tile as tile
    from contextlib import ExitStack

    f32 = mybir.dt.float32
    f16 = mybir.dt.float16
    nc = bacc.Bacc(
        "TRN2", target_bir_lowering=False, debug=False, num_devices=NC,
        num_swdge_queues=4,
    )

    x_d = nc.dram_tensor("x", [N, D], f32, kind="ExternalInput")
    xt_d = nc.dram_tensor("xt", [SLOTS, D], f32, kind="ExternalInput")
    idx_d = nc.dram_tensor("idx16", [P, S_total // 16], mybir.dt.int16, kind="ExternalInput")
    dst_d = nc.dram_tensor("dstl", [P, S_total // 128], f32, kind="ExternalInput")
    wt_d = nc.dram_tensor("wt", [D, D], f32, kind="ExternalInput")
    b_d = nc.dram_tensor("bias", [1, D], f32, kind="ExternalInput")
    out_d = nc.dram_tensor("out", [SLOTS, D], f32, kind="ExternalOutput")

    with tile.TileContext(nc) as tc, ExitStack() as ctx:
        const = ctx.enter_context(tc.tile_pool(name="const", bufs=1))
        gxp = ctx.enter_context(tc.tile_pool(name="gx", bufs=12))
        ohp = ctx.enter_context(tc.tile_pool(name="oh", bufs=12))
        xtp = ctx.enter_context(tc.tile_pool(name="xt", bufs=6))
        htp = ctx.enter_context(tc.tile_pool(name="ht", bufs=6))
        obp = ctx.enter_context(tc.tile_pool(name="ob", bufs=6))
        pag = ctx.enter_context(tc.tile_pool(name="pagg", bufs=6, space="PSUM"))
        pou = ctx.enter_context(tc.tile_pool(name="pout", bufs=2, space="PSUM"))

        idx_t = const.tile([P, S_total // 16], mybir.dt.int16)
        nc.sync.dma_start(out=idx_t[:], in_=idx_d[:])
        dst_t = const.tile([P, S_total // 128], f32)
        nc.sync.dma_start(out=dst_t[:], in_=dst_d[:])
        wt_t = const.tile([D, D], f32)
        nc.sync.dma_start(out=wt_t[:], in_=wt_d[:])
        b_t = const.tile([1, D], f32)
        nc.sync.dma_start(out=b_t[:], in_=b_d[:])
        ones_t = const.tile([1, D], f32)
        nc.vector.memset(ones_t[:], 1.0)
        iota_i = const.tile([P, P], mybir.dt.int32)
        nc.gpsimd.iota(iota_i[:], pattern=[[1, P]], base=0, channel_multiplier=0)
        iota_f = const.tile([P, P], f32)
        nc.vector.tensor_copy(out=iota_f[:], in_=iota_i[:])

        for _rep in range(repeat):
            for (ta, tb) in groups:
                gx_tiles = {}
                oh_tiles = {}
                goff = {}   # q -> first chunk col of (g,q)
                for q in range(NQ):
                    Bgq = int(Btq[ta:tb, q].sum())
                    if Bgq == 0:
                        continue
                    c0 = int(slot_start[ta, q]) // P
                    goff[q] = c0
                    nidx = Bgq * P
                    gx = gxp.tile([P, Bgq, P], f32, tag="gx")
                    nc.gpsimd.dma_gather(
                        gx[:],
                        x_d[q * QROWS : (q + 1) * QROWS, :],
                        idx_t[:, c0 * 8 : c0 * 8 + nidx // 16],
                        nidx,
                        nidx,
                        D,
                        queue_num=q,
                    )
                    oh = ohp.tile([P, Bgq, P], f32, tag="oh")
                    nc.vector.tensor_tensor(
                        out=oh[:],
                        in0=iota_f[:].unsqueeze(1).to_broadcast([P, Bgq, P]),
                        in1=dst_t[:, c0 : c0 + Bgq].unsqueeze(2).to_broadcast([P, Bgq, P]),
                        op=mybir.AluOpType.is_equal,
                    )
                    gx_tiles[q] = gx
                    oh_tiles[q] = oh
                for t in range(ta, tb):
                    chunks = [(q, b) for q in range(NQ) for b in range(int(Btq[t, q]))]
                    xt_t = xtp.tile([P, P], f32, tag="xt")
                    nc.sync.dma_start(out=xt_t[:], in_=xt_d[t * P : (t + 1) * P, :])
                    psum = pag.tile([P, P], f32, space="PSUM", tag="pagg")
                    for i, (q, b) in enumerate(chunks):
                        boff = (int(slot_start[t, q]) // P) - goff[q] + b
                        nc.tensor.matmul(
                            out=psum[:],
                            lhsT=gx_tiles[q][:, boff, :],
                            rhs=oh_tiles[q][:, boff, :],
                            start=(i == 0),
                            stop=(i == len(chunks) - 1),
                        )
                    ht = htp.tile([P, P], f32, tag="ht")
                    if chunks:
                        # h^T = agg^T + x^T (self term)
                        nc.vector.tensor_tensor(
                            out=ht[:], in0=psum[:], in1=xt_t[:],
                            op=mybir.AluOpType.add,
                        )
                    else:
                        nc.vector.tensor_copy(out=ht[:], in_=xt_t[:])
                    po = pou.tile([P, P], f32, space="PSUM", tag="pout")
                    nc.tensor.matmul(out=po[:], lhsT=ht[:], rhs=wt_t[:], start=True, stop=False)
                    nc.tensor.matmul(out=po[:], lhsT=ones_t[:], rhs=b_t[:], start=False, stop=True)
                    ob = obp.tile([P, P], f32, tag="ob")
                    nc.scalar.activation(ob[:], po[:], mybir.ActivationFunctionType.Relu)
                    nc.sync.dma_start(out=out_d[t * P : (t + 1) * P, :], in_=ob[:])
    nc.compile()
    return nc


def _prepare(x, edge_index, W, b, repeat=1):
    x = np.ascontiguousarray(np.asarray(x, dtype=_f32))
    W = np.asarray(W, dtype=_f32)
    b = np.asarray(b, dtype=_f32)
    pre = _preprocess(edge_index)
    nc = _build_program(
        pre["Btq"], pre["slot_start"], pre["S_total"], pre["groups"], repeat=repeat
    )
    wt = np.ascontiguousarray(W.T)
    brow = np.ascontiguousarray(b.reshape(1, D))
    node_of = pre["node_of"]
    in_maps = []
    for c in range(NC):
        nidx = np.where(node_of[c] < 0, 0, node_of[c])
        # x^T per tile: [TPC, D feat, P nodes] flattened to [SLOTS, D]
        xt = np.ascontiguousarray(
            x[nidx].reshape(TPC, P, D).transpose(0, 2, 1).reshape(SLOTS, D)
        )
        in_maps.append(
            {
                "x": x,
                "xt": xt,
                "idx16": np.ascontiguousarray(pre["idx16"][c]),
                "dstl": np.ascontiguousarray(pre["dstl"][c]),
                "wt": wt,
                "bias": brow,
            }
        )
    return nc, in_maps, node_of


def _assemble(results, node_of):
    out = np.empty((N, D), _f32)
    for c in range(NC):
        oc = results[c]["out"]
        m = node_of[c] >= 0
        out[node_of[c][m]] = oc[m]
    return out


def kernel(x, edge_index, W, b):
    from concourse.bass_utils import run_bass_kernel_spmd

    nc, in_maps, node_of = _prepare(x, edge_index, W, b)
    res = run_bass_kernel_spmd(nc, in_maps, core_ids=list(range(NC)))
    return _assemble(res.results, node_of)
